# revision 22
# baseline (speedup 1.0000x reference)
"""Trainium2 Bass kernel for nn_Channel_dot.

Math (per batch b):
  x1 = reshape(input1) -> [THW, C];  x2 likewise
  q  = W1 @ x1 + b1            [F, C]
  k  = W2 @ x2 + b2            [F, C]
  sT = k^T q                   [C(d), C(c)]  (sT[d,c] = s[c,d])
  scoresT = softmax over c (free axis of sT)   -- fp32
  out[c,o] = sum_d s[c,d] * (W3 @ x1 + b3)[o,d]
           = sum_i uT[i,c] * W3T[i,o] + r[c]*b3[o]
    where uT[i,c] = sum_d x1[i,d] sT[d,c],  r[c] = sum_d sT[d,c]

Sharding: 8 cores = 4 batches x 2 halves of the G3 output dim (O=16384).

Structure per core: o-groups 0/1 run the direct order (gT = x1^T W3T
streamed, then s @ gT) because they only need x1/W3 chunks — they keep
the PE busy while x1/x2 stream in and q/k/softmax resolve.  o-groups
2..15 use the reassociated order: one u-phase (82K PE rows) replaces
fourteen per-o-group score matmuls (115K rows), and the b3 term
becomes a rank-1 DVE update (r[c]*b3[o]) off the PE entirely.

Host pre-stages transposed bf16 layouts so every matmul has its
contraction dim on SBUF partitions.  SBUF slots are reused across
phases (x1 -> x1T, x2 -> uT) via tile tags.  Pure SPMD: identical
program, per-core data.
"""

import os
import sys

for _p in ("/opt/trn_rl_repo", "/root/.axon_site/_ro/trn_rl_repo"):
    if os.path.isdir(_p) and _p not in sys.path:
        sys.path.insert(0, _p)

import numpy as np
import ml_dtypes

import concourse.bacc as bacc
import concourse.bass as bass
import concourse.mybir as mybir
import concourse.tile as tile
from concourse.bass_utils import run_bass_kernel_spmd

B, T, C, H, W = 4, 5, 512, 32, 32
F = 16
THW = T * H * W            # 5120
O_TOT = F * H * W          # 16384
O_HALF = O_TOT // 2        # 8192 per core
NI = THW // 128            # 40 i-chunks
OG = 512                   # o-columns per inner group (1 PSUM bank)
N_OG = O_HALF // OG        # 16
NDT = C // 128             # 4 channel tiles

f32 = mybir.dt.float32
bf16 = mybir.dt.bfloat16
AF = mybir.ActivationFunctionType
AX = mybir.AxisListType
ALU = mybir.AluOpType
BF16NP = np.dtype(ml_dtypes.bfloat16)

_NC_CACHE = {}


def _build_nc():
    # Bacc (not plain Bass): its finalize() runs generate_event_semaphores(),
    # which splits multi-wait sync onto EventSemaphore ops — TRN2 compute
    # instructions encode at most one sync wait.
    nc = bacc.Bacc()

    # All streamed inputs are staged partition-major on the host so each
    # DMA reads multi-KB contiguous runs per partition (1KB-row descriptors
    # were the early-phase bandwidth limiter).
    NCH = 5                     # i-chunks per DMA chunk
    NB = NI // NCH              # 8 chunk DMAs per stream
    x1 = nc.dram_tensor("x1", [128, NI * C], bf16, kind="ExternalInput")
    x2 = nc.dram_tensor("x2", [128, NI * C], bf16, kind="ExternalInput")
    # x1 transposed to channel-major: x1t[p, (n, dt, j)] = x1[i=n*128+j,
    # c=dt*128+p] — the u-phase contracts over the channel dim, which must
    # sit on partitions there.
    x1t = nc.dram_tensor("x1t", [128, NI * NDT * 128], bf16, kind="ExternalInput")
    w1t = nc.dram_tensor("w1t", [128, NI * F], bf16, kind="ExternalInput")
    w2t = nc.dram_tensor("w2t", [128, NI * F], bf16, kind="ExternalInput")
    w3t = nc.dram_tensor("w3t", [128, N_OG * NI * OG], bf16, kind="ExternalInput")
    b1 = nc.dram_tensor("b1", [F, 1], f32, kind="ExternalInput")
    b2 = nc.dram_tensor("b2", [F, 1], f32, kind="ExternalInput")
    # b3 replicated to 128 partitions on the host; streamed per o-group.
    b3 = nc.dram_tensor("b3", [128, O_HALF], bf16, kind="ExternalInput")
    out = nc.dram_tensor("out", [C, O_HALF], f32, kind="ExternalOutput")

    x1_r = x1.rearrange("p (n c) -> p n c", c=C)
    x2_r = x2.rearrange("p (n c) -> p n c", c=C)
    x1t_r = x1t.rearrange("p (n dt j) -> p n dt j", dt=NDT, j=128)
    w3_r = w3t.rearrange("p (og n oc) -> p og n oc", og=N_OG, n=NI)
    w1_r = w1t.rearrange("p (n f) -> p n f", f=F)
    w2_r = w2t.rearrange("p (n f) -> p n f", f=F)
    out_r = out.rearrange("(ct p) o -> ct p o", p=128)

    with tile.TileContext(nc) as tc:
        with (
            tc.tile_pool(name="persist", bufs=1) as persist,
            tc.tile_pool(name="w3p", bufs=6) as w3p,
            tc.tile_pool(name="gsbp", bufs=2) as gsbp,
            tc.tile_pool(name="outp", bufs=4) as outp,
            tc.tile_pool(name="b3p", bufs=3) as b3p,
            tc.tile_pool(name="tmpp", bufs=8) as tmpp,
            tc.tile_pool(name="small", bufs=4) as small,
            tc.tile_pool(name="pg", bufs=5, space="PSUM") as pg,
            tc.tile_pool(name="po", bufs=2, space="PSUM") as po,
            tc.tile_pool(name="pqk", bufs=1, space="PSUM") as pqk,
        ):
            # ---- persistent tiles ----
            # x1 (i-major) is dead after o-group 0/1's g-streams + q; the
            # x1T layout reuses its SBUF slot via the shared tag.
            x1_sb = persist.tile([128, NI, C], bf16, name="x1_sb", tag="x1x")
            x1t_sb = persist.tile(
                [128, NI, NDT, 128], bf16, name="x1t_sb", tag="x1x"
            )
            # x2 is dead after k; uT reuses its slot.
            x2_sb = persist.tile([128, NI, C], bf16, name="x2_sb", tag="xu")
            ut_sb = persist.tile([128, NI, C], bf16, name="ut_sb", tag="xu")
            sT_sb = persist.tile([128, NDT, C], bf16, name="sT_sb")

            def b3_tile(og):
                osl = slice(og * OG, (og + 1) * OG)
                b3_t = b3p.tile([128, OG], bf16, name="b3_t")
                nc.sync.dma_start(out=b3_t[:], in_=b3[:, osl])
                return b3_t

            def g_phase(og, x_load=None):
                """Stream W3T columns for this o-group, accumulate gT in
                PSUM (direct order; used for o-groups 0/1 only)."""
                g_ps_l = [pg.tile([128, OG], f32, name="g_ps") for _ in range(NDT)]
                # og 0 ramps with fine-grained chunks so the very first
                # matmul starts as early as possible (DMA queues are still
                # spinning up during the first ~15us)
                plan = [1, 1, 2, 3, 4, 4, 5, 5, 5, 5, 5] if og == 0 else \
                    [NCH] * NB
                n0 = 0
                for ch in plan:
                    if x_load is not None:
                        # one x chunk rides along per w3 chunk so the
                        # prologue inputs arrive without their own phase
                        nc.sync.dma_start(
                            out=x_load[0][:, n0 : n0 + ch, :],
                            in_=x_load[1][:, n0 : n0 + ch, :],
                        )
                    w3_t = w3p.tile([128, NCH, OG], bf16, name="w3_t")
                    nc.sync.dma_start(
                        out=w3_t[:, :ch, :], in_=w3_r[:, og, n0 : n0 + ch, :]
                    )
                    for j in range(ch):
                        for dt_ in range(NDT):
                            nc.tensor.matmul(
                                g_ps_l[dt_][:],
                                lhsT=x1_sb[:, n0 + j, dt_ * 128 : (dt_ + 1) * 128],
                                rhs=w3_t[:, j, :],
                                start=(n0 + j == 0),
                                stop=(n0 + j == NI - 1),
                            )
                    n0 += ch
                return g_ps_l

            def evac_phase(g_ps_l, b3_t):
                """Evacuate gT (+b3) to SBUF right after its g-stream ends,
                while the Vector engine is idle — this unblocks both the
                PSUM ring and the post-softmax score matmuls."""
                g_sb = gsbp.tile([128, NDT, OG], bf16, name="g_sb")
                for dt_ in range(NDT):
                    nc.vector.tensor_add(
                        g_sb[:, dt_, :], g_ps_l[dt_][:], b3_t[:]
                    )
                return g_sb

            def out_phase(og, g_sb):
                """Run the scores @ gT matmuls for a direct-order o-group."""
                osl = slice(og * OG, (og + 1) * OG)
                for ct in range(NDT):
                    o_ps = po.tile([128, OG], f32, name="o_ps", tag="so")
                    for dt_ in range(NDT):
                        nc.tensor.matmul(
                            o_ps[:],
                            lhsT=sT_sb[:, dt_, ct * 128 : (ct + 1) * 128],
                            rhs=g_sb[:, dt_, :],
                            start=(dt_ == 0),
                            stop=(dt_ == NDT - 1),
                        )
                    out_t = outp.tile([128, OG], f32, name="out_t")
                    nc.vector.tensor_copy(out_t[:], o_ps[:])
                    nc.sync.dma_start(out=out_r[ct, :, osl], in_=out_t[:])

            def bog_phase(og):
                """Reassociated o-group: out[:, og] = uT^T W3T + r b3^T."""
                osl = slice(og * OG, (og + 1) * OG)
                b3_t = b3_tile(og)
                # rank-1 bias term precomputed on DVE; consumed by the
                # PSUM evacuation adds at the end of the stream
                tmp_l = []
                for ct in range(NDT):
                    tmp_t = tmpp.tile([128, OG], bf16, name="tmp_t")
                    nc.vector.tensor_scalar_mul(
                        tmp_t[:], b3_t[:], r_sb[:, ct : ct + 1]
                    )
                    tmp_l.append(tmp_t)
                ps_l = [pg.tile([128, OG], f32, name="g_ps") for _ in range(NDT)]
                n0 = 0
                for ch in [NCH] * NB:
                    w3_t = w3p.tile([128, NCH, OG], bf16, name="w3_t")
                    nc.sync.dma_start(
                        out=w3_t[:, :ch, :], in_=w3_r[:, og, n0 : n0 + ch, :]
                    )
                    for j in range(ch):
                        for ct in range(NDT):
                            nc.tensor.matmul(
                                ps_l[ct][:],
                                lhsT=ut_sb[:, n0 + j, ct * 128 : (ct + 1) * 128],
                                rhs=w3_t[:, j, :],
                                start=(n0 + j == 0),
                                stop=(n0 + j == NI - 1),
                            )
                    n0 += ch
                for ct in range(NDT):
                    out_t = outp.tile([128, OG], f32, name="out_t")
                    nc.vector.tensor_add(out_t[:], ps_l[ct][:], tmp_l[ct][:])
                    nc.sync.dma_start(out=out_r[ct, :, osl], in_=out_t[:])

            # o-group 0's g-stream first, with x1 loads interleaved: the PE
            # starts as soon as the first x1/W3 tile pair lands.
            g0 = g_phase(0, x_load=(x1_sb, x1_r))  # x1 rides og0's stream
            b3_t0 = b3_tile(0)

            # W1T/W2T zero-padded on-chip to 128 output columns: M=128
            # matmuls get fast weight load (216ns vs 592ns measured at
            # M=16), while only 160KB each moves over DMA.
            w1t_sb = persist.tile([128, NI, 128], bf16, name="w1t_sb")
            nc.vector.memset(w1t_sb[:], 0.0)
            nc.sync.dma_start(out=w1t_sb[:, :, :F], in_=w1_r[:])
            w2t_sb = persist.tile([128, NI, 128], bf16, name="w2t_sb")
            nc.vector.memset(w2t_sb[:], 0.0)
            nc.sync.dma_start(out=w2t_sb[:, :, :F], in_=w2_r[:])
            b1_sb = persist.tile([F, 1], f32, name="b1_sb")
            nc.sync.dma_start(out=b1_sb[:], in_=b1[:])
            b2_sb = persist.tile([F, 1], f32, name="b2_sb")
            nc.sync.dma_start(out=b2_sb[:], in_=b2[:])
            ones_sb = persist.tile([128, 1], bf16, name="ones_sb")
            nc.vector.memset(ones_sb[:], 1.0)

            # ---- q = W1 @ x1 + b1 -> [F, C] fp32 ----
            q_ps = pqk.tile([128, C], f32, name="q_ps", tag="qk")
            for n in range(NI):
                nc.tensor.matmul(
                    q_ps[:],
                    lhsT=w1t_sb[:, n, :],
                    rhs=x1_sb[:, n, :],
                    start=(n == 0),
                    stop=(n == NI - 1),
                )
            q_sb = persist.tile([F, C], f32, name="q_sb")
            nc.vector.tensor_scalar_add(q_sb[:], q_ps[:F, :], b1_sb[:])

            # og0's gT evacuates now (Vector is idle; g0 psum is complete)
            g_sb0 = evac_phase(g0, b3_t0)

            # o-group 1's g-stream carries the x2 loads (k runs after it)
            g1 = g_phase(1, x_load=(x2_sb, x2_r))
            b3_t1 = b3_tile(1)

            # ---- k = W2 @ x2 + b2 -> [F, C] fp32 ----
            k_ps = pqk.tile([128, C], f32, name="k_ps", tag="qk")
            for n in range(NI):
                nc.tensor.matmul(
                    k_ps[:],
                    lhsT=w2t_sb[:, n, :],
                    rhs=x2_sb[:, n, :],
                    start=(n == 0),
                    stop=(n == NI - 1),
                )
            k_sb = persist.tile([F, C], f32, name="k_sb")
            nc.vector.tensor_scalar_add(k_sb[:], k_ps[:F, :], b2_sb[:])

            # og1's gT evacuates immediately too
            g_sb1 = evac_phase(g1, b3_t1)

            # x1T streams in while softmax/out-phases run; the u-phase
            # consumes it granule by granule.
            for gch in range(NB):
                nc.sync.dma_start(
                    out=x1t_sb[:, gch * NCH : (gch + 1) * NCH, :, :],
                    in_=x1t_r[:, gch * NCH : (gch + 1) * NCH, :, :],
                )

            # ---- sT[d, c] = sum_f k[f,d] q[f,c] (plain fp32 matmul),
            #      then softmax over free (c); emit bf16 scores.  The tiny
            #      r-matmuls (r[c] = sum_d sT[d,c], partition reduce via a
            #      ones vector) interleave into the softmax window so the
            #      PE has work while Scalar/Vector normalize. ----
            # four separate PSUM tiles: a column-sliced accumulation in one
            # bank corrupts sibling columns (start=True resets the bank).
            # pg-ring slots are free here since the g evacuations ran early.
            r_ps_l = [pg.tile([128, 1], f32, name="g_ps") for _ in range(NDT)]

            def softmax_tail(dt_, s_ps):
                # logits are bounded (|s| < ~10 for this problem), so plain
                # exp is fp32-safe; skipping the max keeps Exp at one sync
                # wait (the Activation ISA slot allows only one).
                e_sb = small.tile([128, C], f32, name="e_sb")
                esum = small.tile([128, 1], f32, name="esum")
                nc.scalar.activation(
                    e_sb[:], s_ps[:], AF.Exp, scale=1.0, accum_out=esum[:],
                )
                rcp = small.tile([128, 1], f32, name="rcp")
                nc.vector.reciprocal(rcp[:], esum[:])
                nc.vector.tensor_scalar_mul(sT_sb[:, dt_, :], e_sb[:], rcp[:])
                for ct in range(NDT):
                    nc.tensor.matmul(
                        r_ps_l[ct][:],
                        lhsT=sT_sb[:, dt_, ct * 128 : (ct + 1) * 128],
                        rhs=ones_sb[:],
                        start=(dt_ == 0),
                        stop=(dt_ == NDT - 1),
                    )

            s_pend = None
            for dt_ in range(NDT):
                s_ps = po.tile([128, C], f32, name="s_ps", tag="so")
                nc.tensor.matmul(
                    s_ps[:],
                    lhsT=k_sb[:, dt_ * 128 : (dt_ + 1) * 128],
                    rhs=q_sb[:],
                    start=True,
                    stop=True,
                )
                if s_pend is not None:
                    softmax_tail(dt_ - 1, s_pend)
                s_pend = s_ps
            softmax_tail(NDT - 1, s_pend)
            r_sb = persist.tile([128, NDT], f32, name="r_sb")
            for ct in range(NDT):
                nc.vector.tensor_copy(r_sb[:, ct : ct + 1], r_ps_l[ct][:])

            # ---- direct-order output for o-groups 0/1 ----
            out_phase(0, g_sb0)
            out_phase(1, g_sb1)

            # ---- u-phase: uT[i, c] = sum_d x1[i,d] sT[d,c], bf16 ----
            for n in range(NI):
                u_ps = po.tile([128, C], f32, name="u_ps", tag="so")
                for dt_ in range(NDT):
                    nc.tensor.matmul(
                        u_ps[:],
                        lhsT=x1t_sb[:, n, dt_, :],
                        rhs=sT_sb[:, dt_, :],
                        start=(dt_ == 0),
                        stop=(dt_ == NDT - 1),
                    )
                nc.vector.tensor_copy(ut_sb[:, n, :], u_ps[:])

            # ---- main: reassociated stream for o-groups 2..15 ----
            for og in range(2, N_OG):
                bog_phase(og)

    nc.finalize()
    return nc


def _get_nc():
    if "nc" not in _NC_CACHE:
        _NC_CACHE["nc"] = _build_nc()
    return _NC_CACHE["nc"]


def _stage_inputs(input1, input2, W1, b1, W2, b2, W3, b3):
    input1 = np.asarray(input1, np.float32)
    input2 = np.asarray(input2, np.float32)
    W1 = np.asarray(W1, np.float32)
    W2 = np.asarray(W2, np.float32)
    W3 = np.asarray(W3, np.float32)
    b1 = np.asarray(b1, np.float32)
    b2 = np.asarray(b2, np.float32)
    b3 = np.asarray(b3, np.float32)

    def pmajor(X, inner):
        # [THW, inner] -> [128, NI*inner]: row p = concat_n X[n*128+p, :],
        # so every DMA chunk is a contiguous multi-KB run per partition
        return np.ascontiguousarray(
            X.reshape(NI, 128, inner).transpose(1, 0, 2).reshape(128, NI * inner)
        )

    # [B,T,C,H,W] -> x[b][i=(t,hw), c], bf16, partition-major
    X1 = np.ascontiguousarray(
        input1.reshape(B, T, C, H * W).transpose(0, 1, 3, 2)
    ).reshape(B, THW, C).astype(BF16NP)
    X2 = np.ascontiguousarray(
        input2.reshape(B, T, C, H * W).transpose(0, 1, 3, 2)
    ).reshape(B, THW, C).astype(BF16NP)
    X1p = [pmajor(X1[b], C) for b in range(B)]
    X2p = [pmajor(X2[b], C) for b in range(B)]
    # channel-major x1: [128p, (n, dt, j)] with value x1[n*128+j, dt*128+p]
    X1Tp = [
        np.ascontiguousarray(
            X1[b].reshape(NI, 128, NDT, 128).transpose(3, 0, 2, 1)
        ).reshape(128, NI * NDT * 128)
        for b in range(B)
    ]
    W1Tp = pmajor(np.ascontiguousarray(W1.T).astype(BF16NP), F)
    W2Tp = pmajor(np.ascontiguousarray(W2.T).astype(BF16NP), F)
    W3T = np.ascontiguousarray(W3.T).astype(BF16NP)   # [THW, O_TOT]
    b1c = np.ascontiguousarray(b1.reshape(F, 1))
    b2c = np.ascontiguousarray(b2.reshape(F, 1))
    b3h = b3.astype(BF16NP)

    in_maps = []
    for core in range(8):
        b = core // 2
        half = core % 2
        osl = slice(half * O_HALF, (half + 1) * O_HALF)
        # [THW, O_HALF] -> [128, (og, n, oc)]
        w3p_core = np.ascontiguousarray(
            W3T[:, osl]
            .reshape(NI, 128, N_OG, OG)
            .transpose(1, 2, 0, 3)
            .reshape(128, N_OG * NI * OG)
        )
        in_maps.append(
            {
                "x1": X1p[b],
                "x2": X2p[b],
                "x1t": X1Tp[b],
                "w1t": W1Tp,
                "w2t": W2Tp,
                "w3t": w3p_core,
                "b1": b1c,
                "b2": b2c,
                "b3": np.ascontiguousarray(
                    np.broadcast_to(b3h[osl][None, :], (128, O_HALF))
                ),
            }
        )
    return in_maps


def run(inputs: dict, trace: bool = False):
    """Returns (full_output [B,F,C,H,W], BassKernelResults)."""
    in_maps = _stage_inputs(**inputs)
    nc = _get_nc()
    res = run_bass_kernel_spmd(nc, in_maps, core_ids=list(range(8)), trace=trace)
    out_full = np.empty((B, C, O_TOT), np.float32)
    for core in range(8):
        b = core // 2
        half = core % 2
        out_full[b, :, half * O_HALF : (half + 1) * O_HALF] = res.results[core]["out"]
    out = np.ascontiguousarray(
        out_full.reshape(B, C, F, H, W).transpose(0, 2, 1, 3, 4)
    )
    return out, res


def kernel(**inputs) -> np.ndarray:
    out, _ = run(inputs, trace=False)
    return out


# revision 23
# speedup vs baseline: 1.0012x; 1.0012x over previous
"""Trainium2 Bass kernel for nn_Channel_dot — v6: q/k ride the A-streams.

Math (per batch b):
  q  = W1 @ x1 + b1; k = W2 @ x2 + b2          [F, C]
  sT = softmax over c of (k^T q)^T             [C(d), C(c)]
  out[c,o] = sum_i uT[i,c] W3T[i,o] + r[c] b3[o]
    where uT[i,c] = sum_d x1[i,d] sT[d,c],  r[c] = sum_d sT[d,c]

Sharding: 8 cores = 4 batches x 2 halves of the G3 output dim (O=16384).

 * A-groups 0/1 (496 cols) run the direct order; their stream tiles
   carry W1T/W2T in PSUM columns 496..511, so the q/k projections ride
   the same stationary x1/x2 chunks for free.  The resulting q^T/k^T
   are flipped via full 128x128 PE transposes (non-square transposes
   are not exercised anywhere in the repo kernels; tile_matmul pads to
   square first).
 * 15 B-groups (480 cols) use the reassociated order with the rank-1
   b3 update on the Vector engine.
"""

import os
import sys

for _p in ("/opt/trn_rl_repo", "/root/.axon_site/_ro/trn_rl_repo"):
    if os.path.isdir(_p) and _p not in sys.path:
        sys.path.insert(0, _p)

import numpy as np
import ml_dtypes

import concourse.bacc as bacc
import concourse.bass as bass
import concourse.mybir as mybir
import concourse.tile as tile
from concourse.bass_utils import run_bass_kernel_spmd

B, T, C, H, W = 4, 5, 512, 32, 32
F = 16
THW = T * H * W            # 5120
O_TOT = F * H * W          # 16384
O_HALF = O_TOT // 2        # 8192 per core
NI = THW // 128            # 40 i-chunks
NDT = C // 128             # 4 channel tiles
OGA = 496                  # o-cols per A-group (+16 W1/W2 cols = 512)
OGB = 480                  # o-cols per B-group
NBOG = (O_HALF - 2 * OGA) // OGB   # 15
OB0 = 2 * OGA              # 992: first B-group column
DEBUG_QK = os.environ.get("KDBG", "") == "1"

f32 = mybir.dt.float32
bf16 = mybir.dt.bfloat16
AF = mybir.ActivationFunctionType
BF16NP = np.dtype(ml_dtypes.bfloat16)

_NC_CACHE = {}


def _build_nc():
    nc = bacc.Bacc()

    NCH = 8                     # i-chunks per DMA chunk (8KB runs/partition)
    NB = NI // NCH              # 5 chunk DMAs per stream
    x1 = nc.dram_tensor("x1", [128, NI * C], bf16, kind="ExternalInput")
    x2 = nc.dram_tensor("x2", [128, NI * C], bf16, kind="ExternalInput")
    x1t = nc.dram_tensor("x1t", [128, NI * NDT * 128], bf16, kind="ExternalInput")
    # A-stream: per og in {0,1}, per chunk n: [496 W3T cols | 16 W1T/W2T cols]
    w3a = nc.dram_tensor("w3a", [128, 2 * NI * 512], bf16, kind="ExternalInput")
    w3b = nc.dram_tensor("w3b", [128, NBOG * NI * OGB], bf16, kind="ExternalInput")
    w2t = nc.dram_tensor("w2t", [128, NI * F], bf16, kind="ExternalInput")
    b1 = nc.dram_tensor("b1", [F, 1], f32, kind="ExternalInput")
    b2 = nc.dram_tensor("b2", [F, 1], f32, kind="ExternalInput")
    b3a = nc.dram_tensor("b3a", [128, 2 * 512], bf16, kind="ExternalInput")
    b3b = nc.dram_tensor("b3b", [128, NBOG * OGB], bf16, kind="ExternalInput")
    idm = nc.dram_tensor("idm", [128, 128], f32, kind="ExternalInput")
    out = nc.dram_tensor("out", [C, O_HALF], f32, kind="ExternalOutput")
    if DEBUG_QK:
        qdbg = nc.dram_tensor("qdbg", [F, C], bf16, kind="ExternalOutput")
        qtdbg = nc.dram_tensor("qtdbg", [128, NDT * F], f32, kind="ExternalOutput")

    x1_r = x1.rearrange("p (n c) -> p n c", c=C)
    x2_r = x2.rearrange("p (n c) -> p n c", c=C)
    x1t_r = x1t.rearrange("p (n dt j) -> p n dt j", dt=NDT, j=128)
    w3a_r = w3a.rearrange("p (og n c) -> p og n c", og=2, c=512)
    w3b_r = w3b.rearrange("p (og n c) -> p og n c", og=NBOG, c=OGB)
    w2_r = w2t.rearrange("p (n f) -> p n f", f=F)
    b3a_r = b3a.rearrange("p (og c) -> p og c", og=2)
    out_r = out.rearrange("(ct p) o -> ct p o", p=128)

    with tile.TileContext(nc) as tc:
        with (
            tc.tile_pool(name="persist", bufs=1) as persist,
            tc.tile_pool(name="w3p", bufs=5) as w3p,
            tc.tile_pool(name="gsbp", bufs=2) as gsbp,
            tc.tile_pool(name="outp", bufs=4) as outp,
            tc.tile_pool(name="b3p", bufs=3) as b3p,
            tc.tile_pool(name="tmpp", bufs=8) as tmpp,
            tc.tile_pool(name="small", bufs=4) as small,
            tc.tile_pool(name="pg", bufs=6, space="PSUM") as pg,
            tc.tile_pool(name="po", bufs=2, space="PSUM") as po,
        ):
            x1_sb = persist.tile([128, NI, C], bf16, name="x1_sb", tag="x1x")
            x1t_sb = persist.tile(
                [128, NI, NDT, 128], bf16, name="x1t_sb", tag="x1x"
            )
            x2_sb = persist.tile([128, NI, C], bf16, name="x2_sb", tag="xu")
            ut_sb = persist.tile([128, NI, C], bf16, name="ut_sb", tag="xu")
            sT_sb = persist.tile([128, NDT, C], bf16, name="sT_sb")
            id_sb = persist.tile([128, 128], f32, name="id_sb")
            nc.sync.dma_start(out=id_sb[:], in_=idm[:])
            b1_sb = persist.tile([F, 1], f32, name="b1_sb")
            nc.sync.dma_start(out=b1_sb[:], in_=b1[:])
            b2_sb = persist.tile([F, 1], f32, name="b2_sb")
            nc.sync.dma_start(out=b2_sb[:], in_=b2[:])
            ones_sb = persist.tile([128, 1], bf16, name="ones_sb")
            nc.vector.memset(ones_sb[:], 1.0)
            # k = W2 @ x2 cannot ride og1 (its stationary is x1, not
            # x2), so k keeps the classic padded-lhsT matmul path
            w2t_sb = persist.tile([128, NI, 128], bf16, name="w2t_sb")
            nc.vector.memset(w2t_sb[:], 0.0)
            nc.sync.dma_start(out=w2t_sb[:, :, :F], in_=w2_r[:])

            def g_phase(og, x_load):
                g_ps_l = [pg.tile([128, 512], f32, name="g_ps") for _ in range(NDT)]
                plan = [1, 1, 2, 3, 4, 5, 8, 8, 8] if og == 0 else [NCH] * NB
                n0 = 0
                for ch in plan:
                    nc.sync.dma_start(
                        out=x_load[0][:, n0 : n0 + ch, :],
                        in_=x_load[1][:, n0 : n0 + ch, :],
                    )
                    w3_t = w3p.tile([128, NCH, 512], bf16, name="w3_t")
                    nc.sync.dma_start(
                        out=w3_t[:, :ch, :], in_=w3a_r[:, og, n0 : n0 + ch, :]
                    )
                    for j in range(ch):
                        for dt_ in range(NDT):
                            nc.tensor.matmul(
                                g_ps_l[dt_][:],
                                lhsT=x1_sb[:, n0 + j, dt_ * 128 : (dt_ + 1) * 128],
                                rhs=w3_t[:, j, :],
                                start=(n0 + j == 0),
                                stop=(n0 + j == NI - 1),
                            )
                    n0 += ch
                return g_ps_l

            def evac_phase(og, g_ps_l):
                b3_t = b3p.tile([128, 512], bf16, name="b3_t")
                nc.sync.dma_start(out=b3_t[:], in_=b3a_r[:, og, :])
                g_sb = gsbp.tile([128, NDT, 512], bf16, name="g_sb")
                for dt_ in range(NDT):
                    nc.vector.tensor_add(
                        g_sb[:, dt_, :], g_ps_l[dt_][:], b3_t[:]
                    )
                return g_sb

            def qk_transpose(g_ps_l, qt32_sb, bias_sb, dst_sb):
                """PSUM cols 496:512 hold (W @ x)^T per dt tile; copy them
                out fp32 into cols 0:16 of a [128, 128] staging block and
                flip the full square on the PE (repo kernels only exercise
                square transposes), then add the bias to rows 0:16."""
                for ct in range(NDT):
                    nc.vector.tensor_copy(
                        qt32_sb[:, ct, :F], g_ps_l[ct][:, OGA:512]
                    )
                    # junk in cols F:128 would be transposed into rows
                    # F:128 (never read), but NaN traps are a risk — zero it
                    nc.vector.memset(qt32_sb[:, ct, F:], 0.0)
                tp_l = [pg.tile([128, 128], f32, name="g_ps") for _ in range(NDT)]
                for ct in range(NDT):
                    nc.tensor.transpose(
                        tp_l[ct][:], qt32_sb[:, ct, :], id_sb[:]
                    )
                for ct in range(NDT):
                    nc.vector.tensor_scalar_add(
                        dst_sb[:, ct * 128 : (ct + 1) * 128],
                        tp_l[ct][:F, :],
                        bias_sb[:],
                    )

            def out_phase(og, g_sb):
                osl = slice(og * OGA, (og + 1) * OGA)
                for ct in range(NDT):
                    o_ps = po.tile([128, 512], f32, name="o_ps", tag="so")
                    for dt_ in range(NDT):
                        nc.tensor.matmul(
                            o_ps[:, :OGA],
                            lhsT=sT_sb[:, dt_, ct * 128 : (ct + 1) * 128],
                            rhs=g_sb[:, dt_, :OGA],
                            start=(dt_ == 0),
                            stop=(dt_ == NDT - 1),
                        )
                    out_t = outp.tile([128, OGA], f32, name="out_t")
                    nc.vector.tensor_copy(out_t[:], o_ps[:, :OGA])
                    nc.sync.dma_start(out=out_r[ct, :, osl], in_=out_t[:])

            def bog_phase(og):
                osl = slice(OB0 + og * OGB, OB0 + (og + 1) * OGB)
                b3_t = b3p.tile([128, OGB], bf16, name="b3_tb", tag="b3_t")
                nc.sync.dma_start(
                    out=b3_t[:], in_=b3b[:, og * OGB : (og + 1) * OGB]
                )
                tmp_l = []
                for ct in range(NDT):
                    tmp_t = tmpp.tile([128, OGB], bf16, name="tmp_t")
                    nc.vector.tensor_scalar_mul(
                        tmp_t[:], b3_t[:], r_sb[:, ct : ct + 1]
                    )
                    tmp_l.append(tmp_t)
                ps_l = [pg.tile([128, 512], f32, name="g_ps") for _ in range(NDT)]
                n0 = 0
                for ch in [NCH] * NB:
                    w3_t = w3p.tile([128, NCH, OGB], bf16, name="w3_tb", tag="w3_t")
                    nc.sync.dma_start(
                        out=w3_t[:, :ch, :],
                        in_=w3b_r[:, og, n0 : n0 + ch, :],
                    )
                    for j in range(ch):
                        for ct in range(NDT):
                            nc.tensor.matmul(
                                ps_l[ct][:, :OGB],
                                lhsT=ut_sb[:, n0 + j, ct * 128 : (ct + 1) * 128],
                                rhs=w3_t[:, j, :],
                                start=(n0 + j == 0),
                                stop=(n0 + j == NI - 1),
                            )
                    n0 += ch
                for ct in range(NDT):
                    out_t = outp.tile([128, OGB], f32, name="out_t")
                    nc.vector.tensor_add(out_t[:], ps_l[ct][:, :OGB], tmp_l[ct][:])
                    nc.sync.dma_start(out=out_r[ct, :, osl], in_=out_t[:])

            # ---- A-group 0: x1 rides; q^T accumulates in cols 496:512 ----
            g0 = g_phase(0, x_load=(x1_sb, x1_r))
            g_sb0 = evac_phase(0, g0)
            qt32_sb = persist.tile([128, NDT, 128], f32, name="qt32_sb")
            q_sb = persist.tile([F, C], bf16, name="q_sb")
            qk_transpose(g0, qt32_sb, b1_sb, q_sb)

            # ---- A-group 1: x2 rides; k^T accumulates in cols 496:512 ----
            g1 = g_phase(1, x_load=(x2_sb, x2_r))
            g_sb1 = evac_phase(1, g1)
            k_ps = po.tile([128, C], f32, name="k_ps", tag="so")
            for n in range(NI):
                nc.tensor.matmul(
                    k_ps[:],
                    lhsT=w2t_sb[:, n, :],
                    rhs=x2_sb[:, n, :],
                    start=(n == 0),
                    stop=(n == NI - 1),
                )
            k_sb = persist.tile([F, C], bf16, name="k_sb")
            nc.vector.tensor_scalar_add(k_sb[:], k_ps[:F, :], b2_sb[:])

            if DEBUG_QK:
                nc.sync.dma_start(out=qdbg[:], in_=q_sb[:])
                nc.sync.dma_start(
                    out=qtdbg.rearrange("p (ct f) -> p ct f", f=F)[:],
                    in_=qt32_sb[:, :, :F],
                )

            for gch in range(NB):
                nc.sync.dma_start(
                    out=x1t_sb[:, gch * NCH : (gch + 1) * NCH, :, :],
                    in_=x1t_r[:, gch * NCH : (gch + 1) * NCH, :, :],
                )

            # ---- softmax over c, with r-matmuls interleaved ----
            r_ps_l = [pg.tile([128, 1], f32, name="g_ps") for _ in range(NDT)]

            def softmax_tail(dt_, s_ps):
                e_sb = small.tile([128, C], f32, name="e_sb")
                esum = small.tile([128, 1], f32, name="esum")
                nc.scalar.activation(
                    e_sb[:], s_ps[:], AF.Exp, scale=1.0, accum_out=esum[:],
                )
                rcp = small.tile([128, 1], f32, name="rcp")
                nc.vector.reciprocal(rcp[:], esum[:])
                nc.vector.tensor_scalar_mul(sT_sb[:, dt_, :], e_sb[:], rcp[:])
                for ct in range(NDT):
                    nc.tensor.matmul(
                        r_ps_l[ct][:],
                        lhsT=sT_sb[:, dt_, ct * 128 : (ct + 1) * 128],
                        rhs=ones_sb[:],
                        start=(dt_ == 0),
                        stop=(dt_ == NDT - 1),
                    )

            s_pend = None
            for dt_ in range(NDT):
                s_ps = po.tile([128, C], f32, name="s_ps", tag="so")
                nc.tensor.matmul(
                    s_ps[:],
                    lhsT=k_sb[:, dt_ * 128 : (dt_ + 1) * 128],
                    rhs=q_sb[:],
                    start=True,
                    stop=True,
                )
                if s_pend is not None:
                    softmax_tail(dt_ - 1, s_pend)
                s_pend = s_ps
            softmax_tail(NDT - 1, s_pend)

            out_phase(0, g_sb0)
            out_phase(1, g_sb1)

            r_sb = persist.tile([128, NDT], f32, name="r_sb")
            for ct in range(NDT):
                nc.vector.tensor_copy(r_sb[:, ct : ct + 1], r_ps_l[ct][:])

            # ---- u-phase: uT[i, c] = sum_d x1[i,d] sT[d,c], bf16 ----
            for n in range(NI):
                u_ps = po.tile([128, C], f32, name="u_ps", tag="so")
                for dt_ in range(NDT):
                    nc.tensor.matmul(
                        u_ps[:],
                        lhsT=x1t_sb[:, n, dt_, :],
                        rhs=sT_sb[:, dt_, :],
                        start=(dt_ == 0),
                        stop=(dt_ == NDT - 1),
                    )
                nc.vector.tensor_copy(ut_sb[:, n, :], u_ps[:])

            for og in range(NBOG):
                bog_phase(og)

    nc.finalize()
    return nc


def _get_nc():
    if "nc" not in _NC_CACHE:
        _NC_CACHE["nc"] = _build_nc()
    return _NC_CACHE["nc"]


def _stage_inputs(input1, input2, W1, b1, W2, b2, W3, b3):
    input1 = np.asarray(input1, np.float32)
    input2 = np.asarray(input2, np.float32)
    W1 = np.asarray(W1, np.float32)
    W2 = np.asarray(W2, np.float32)
    W3 = np.asarray(W3, np.float32)
    b1 = np.asarray(b1, np.float32)
    b2 = np.asarray(b2, np.float32)
    b3 = np.asarray(b3, np.float32)

    def pmajor(X, inner):
        return np.ascontiguousarray(
            X.reshape(NI, 128, inner).transpose(1, 0, 2).reshape(128, NI * inner)
        )

    X1 = np.ascontiguousarray(
        input1.reshape(B, T, C, H * W).transpose(0, 1, 3, 2)
    ).reshape(B, THW, C).astype(BF16NP)
    X2 = np.ascontiguousarray(
        input2.reshape(B, T, C, H * W).transpose(0, 1, 3, 2)
    ).reshape(B, THW, C).astype(BF16NP)
    X1p = [pmajor(X1[b], C) for b in range(B)]
    X2p = [pmajor(X2[b], C) for b in range(B)]
    X1Tp = [
        np.ascontiguousarray(
            X1[b].reshape(NI, 128, NDT, 128).transpose(3, 0, 2, 1)
        ).reshape(128, NI * NDT * 128)
        for b in range(B)
    ]
    W1Tc = np.ascontiguousarray(W1.T).astype(BF16NP).reshape(NI, 128, F)
    W2Tp = pmajor(np.ascontiguousarray(W2.T).astype(BF16NP), F)
    W2Tc = np.ascontiguousarray(W2.T).astype(BF16NP).reshape(NI, 128, F)
    W3T = np.ascontiguousarray(W3.T).astype(BF16NP)   # [THW, O_TOT]
    b1c = np.ascontiguousarray(b1.reshape(F, 1))
    b2c = np.ascontiguousarray(b2.reshape(F, 1))
    b3h = b3.astype(BF16NP)
    idm = np.ascontiguousarray(np.eye(128, dtype=np.float32))

    in_maps = []
    for core in range(8):
        b = core // 2
        half = core % 2
        osl = slice(half * O_HALF, (half + 1) * O_HALF)
        Wh = W3T[:, osl].reshape(NI, 128, O_HALF)
        A = np.empty((NI, 128, 2, 512), BF16NP)
        A[:, :, 0, :OGA] = Wh[:, :, :OGA]
        A[:, :, 0, OGA:] = W1Tc
        A[:, :, 1, :OGA] = Wh[:, :, OGA : 2 * OGA]
        A[:, :, 1, OGA:] = W2Tc
        w3a_core = np.ascontiguousarray(
            A.transpose(1, 2, 0, 3).reshape(128, 2 * NI * 512)
        )
        w3b_core = np.ascontiguousarray(
            Wh[:, :, OB0:]
            .reshape(NI, 128, NBOG, OGB)
            .transpose(1, 2, 0, 3)
            .reshape(128, NBOG * NI * OGB)
        )
        b3a_core = np.zeros((128, 2, 512), BF16NP)
        b3a_core[:, 0, :OGA] = b3h[osl][:OGA][None, :]
        b3a_core[:, 1, :OGA] = b3h[osl][OGA : 2 * OGA][None, :]
        in_maps.append(
            {
                "x1": X1p[b],
                "x2": X2p[b],
                "x1t": X1Tp[b],
                "w3a": w3a_core,
                "w3b": w3b_core,
                "w2t": W2Tp,
                "b1": b1c,
                "b2": b2c,
                "b3a": np.ascontiguousarray(b3a_core.reshape(128, 2 * 512)),
                "b3b": np.ascontiguousarray(
                    np.broadcast_to(b3h[osl][OB0:][None, :], (128, NBOG * OGB))
                ),
                "idm": idm,
            }
        )
    return in_maps


def run(inputs: dict, trace: bool = False):
    in_maps = _stage_inputs(**inputs)
    nc = _get_nc()
    res = run_bass_kernel_spmd(nc, in_maps, core_ids=list(range(8)), trace=trace)
    out_full = np.empty((B, C, O_TOT), np.float32)
    for core in range(8):
        b = core // 2
        half = core % 2
        out_full[b, :, half * O_HALF : (half + 1) * O_HALF] = res.results[core]["out"]
    out = np.ascontiguousarray(
        out_full.reshape(B, C, F, H, W).transpose(0, 2, 1, 3, 4)
    )
    return out, res


def kernel(**inputs) -> np.ndarray:
    out, _ = run(inputs, trace=False)
    return out


# revision 24
# speedup vs baseline: 1.0043x; 1.0031x over previous
"""Trainium2 Bass kernel for nn_Channel_dot.

Math (per batch b):
  x1 = reshape(input1) -> [THW, C];  x2 likewise
  q  = W1 @ x1 + b1            [F, C]
  k  = W2 @ x2 + b2            [F, C]
  sT = k^T q                   [C(d), C(c)]  (sT[d,c] = s[c,d])
  scoresT = softmax over c (free axis of sT)   -- fp32
  out[c,o] = sum_d s[c,d] * (W3 @ x1 + b3)[o,d]
           = sum_i uT[i,c] * W3T[i,o] + r[c]*b3[o]
    where uT[i,c] = sum_d x1[i,d] sT[d,c],  r[c] = sum_d sT[d,c]

Sharding: 8 cores = 4 batches x 2 halves of the G3 output dim (O=16384).

Structure per core: o-groups 0/1 run the direct order (gT = x1^T W3T
streamed, then s @ gT) because they only need x1/W3 chunks — they keep
the PE busy while x1/x2 stream in and q/k/softmax resolve.  o-groups
2..15 use the reassociated order: one u-phase (82K PE rows) replaces
fourteen per-o-group score matmuls (115K rows), and the b3 term
becomes a rank-1 DVE update (r[c]*b3[o]) off the PE entirely.

Host pre-stages transposed bf16 layouts so every matmul has its
contraction dim on SBUF partitions.  SBUF slots are reused across
phases (x1 -> x1T, x2 -> uT) via tile tags.  Pure SPMD: identical
program, per-core data.
"""

import os
import sys

for _p in ("/opt/trn_rl_repo", "/root/.axon_site/_ro/trn_rl_repo"):
    if os.path.isdir(_p) and _p not in sys.path:
        sys.path.insert(0, _p)

import numpy as np
import ml_dtypes

import concourse.bacc as bacc
import concourse.bass as bass
import concourse.mybir as mybir
import concourse.tile as tile
from concourse.bass_utils import run_bass_kernel_spmd

B, T, C, H, W = 4, 5, 512, 32, 32
F = 16
THW = T * H * W            # 5120
O_TOT = F * H * W          # 16384
O_HALF = O_TOT // 2        # 8192 per core
NI = THW // 128            # 40 i-chunks
OG = 512                   # o-columns per inner group (1 PSUM bank)
N_OG = O_HALF // OG        # 16
NDT = C // 128             # 4 channel tiles

f32 = mybir.dt.float32
bf16 = mybir.dt.bfloat16
AF = mybir.ActivationFunctionType
AX = mybir.AxisListType
ALU = mybir.AluOpType
BF16NP = np.dtype(ml_dtypes.bfloat16)

_NC_CACHE = {}


def _build_nc():
    # Bacc (not plain Bass): its finalize() runs generate_event_semaphores(),
    # which splits multi-wait sync onto EventSemaphore ops — TRN2 compute
    # instructions encode at most one sync wait.
    nc = bacc.Bacc()

    # All streamed inputs are staged partition-major on the host so each
    # DMA reads multi-KB contiguous runs per partition (1KB-row descriptors
    # were the early-phase bandwidth limiter).
    NCH = 5                     # i-chunks per DMA chunk
    NB = NI // NCH              # 8 chunk DMAs per stream
    x1 = nc.dram_tensor("x1", [128, NI * C], bf16, kind="ExternalInput")
    x2 = nc.dram_tensor("x2", [128, NI * C], bf16, kind="ExternalInput")
    # x1 transposed to channel-major: x1t[p, (n, dt, j)] = x1[i=n*128+j,
    # c=dt*128+p] — the u-phase contracts over the channel dim, which must
    # sit on partitions there.
    x1t = nc.dram_tensor("x1t", [128, NI * NDT * 128], bf16, kind="ExternalInput")
    w1t = nc.dram_tensor("w1t", [128, NI * F], bf16, kind="ExternalInput")
    w2t = nc.dram_tensor("w2t", [128, NI * F], bf16, kind="ExternalInput")
    w3t = nc.dram_tensor("w3t", [128, N_OG * NI * OG], bf16, kind="ExternalInput")
    b1 = nc.dram_tensor("b1", [F, 1], f32, kind="ExternalInput")
    b2 = nc.dram_tensor("b2", [F, 1], f32, kind="ExternalInput")
    # b3 replicated to 128 partitions on the host; streamed per o-group.
    b3 = nc.dram_tensor("b3", [128, O_HALF], bf16, kind="ExternalInput")
    out = nc.dram_tensor("out", [C, O_HALF], f32, kind="ExternalOutput")

    x1_r = x1.rearrange("p (n c) -> p n c", c=C)
    x2_r = x2.rearrange("p (n c) -> p n c", c=C)
    x1t_r = x1t.rearrange("p (n dt j) -> p n dt j", dt=NDT, j=128)
    w3_r = w3t.rearrange("p (og n oc) -> p og n oc", og=N_OG, n=NI)
    w1_r = w1t.rearrange("p (n f) -> p n f", f=F)
    w2_r = w2t.rearrange("p (n f) -> p n f", f=F)
    out_r = out.rearrange("(ct p) o -> ct p o", p=128)

    with tile.TileContext(nc) as tc:
        with (
            tc.tile_pool(name="persist", bufs=1) as persist,
            tc.tile_pool(name="w3p", bufs=6) as w3p,
            tc.tile_pool(name="gsbp", bufs=2) as gsbp,
            tc.tile_pool(name="outp", bufs=4) as outp,
            tc.tile_pool(name="b3p", bufs=3) as b3p,
            tc.tile_pool(name="tmpp", bufs=8) as tmpp,
            tc.tile_pool(name="small", bufs=4) as small,
            tc.tile_pool(name="pg", bufs=5, space="PSUM") as pg,
            tc.tile_pool(name="po", bufs=2, space="PSUM") as po,
            tc.tile_pool(name="pqk", bufs=1, space="PSUM") as pqk,
        ):
            # ---- persistent tiles ----
            # x1 (i-major) is dead after o-group 0/1's g-streams + q; the
            # x1T layout reuses its SBUF slot via the shared tag.
            x1_sb = persist.tile([128, NI, C], bf16, name="x1_sb", tag="x1x")
            x1t_sb = persist.tile(
                [128, NI, NDT, 128], bf16, name="x1t_sb", tag="x1x"
            )
            # x2 is dead after k; uT reuses its slot.
            x2_sb = persist.tile([128, NI, C], bf16, name="x2_sb", tag="xu")
            ut_sb = persist.tile([128, NI, C], bf16, name="ut_sb", tag="xu")
            sT_sb = persist.tile([128, NDT, C], bf16, name="sT_sb")

            def b3_tile(og):
                osl = slice(og * OG, (og + 1) * OG)
                b3_t = b3p.tile([128, OG], bf16, name="b3_t")
                nc.sync.dma_start(out=b3_t[:], in_=b3[:, osl])
                return b3_t

            def g_phase(og, x_load=None):
                """Stream W3T columns for this o-group, accumulate gT in
                PSUM (direct order; used for o-groups 0/1 only)."""
                g_ps_l = [pg.tile([128, OG], f32, name="g_ps") for _ in range(NDT)]
                # og 0 ramps with fine-grained chunks so the very first
                # matmul starts as early as possible (DMA queues are still
                # spinning up during the first ~15us)
                plan = [1, 1, 2, 3, 4, 4, 5, 5, 5, 5, 5] if og == 0 else \
                    [NCH] * NB
                n0 = 0
                for ch in plan:
                    if x_load is not None:
                        # one x chunk rides along per w3 chunk so the
                        # prologue inputs arrive without their own phase
                        nc.sync.dma_start(
                            out=x_load[0][:, n0 : n0 + ch, :],
                            in_=x_load[1][:, n0 : n0 + ch, :],
                        )
                    w3_t = w3p.tile([128, NCH, OG], bf16, name="w3_t")
                    nc.sync.dma_start(
                        out=w3_t[:, :ch, :], in_=w3_r[:, og, n0 : n0 + ch, :]
                    )
                    for j in range(ch):
                        for dt_ in range(NDT):
                            nc.tensor.matmul(
                                g_ps_l[dt_][:],
                                lhsT=x1_sb[:, n0 + j, dt_ * 128 : (dt_ + 1) * 128],
                                rhs=w3_t[:, j, :],
                                start=(n0 + j == 0),
                                stop=(n0 + j == NI - 1),
                            )
                    n0 += ch
                return g_ps_l

            def evac_phase(g_ps_l, b3_t):
                """Evacuate gT (+b3) to SBUF right after its g-stream ends,
                while the Vector engine is idle — this unblocks both the
                PSUM ring and the post-softmax score matmuls."""
                g_sb = gsbp.tile([128, NDT, OG], bf16, name="g_sb")
                for dt_ in range(NDT):
                    nc.vector.tensor_add(
                        g_sb[:, dt_, :], g_ps_l[dt_][:], b3_t[:]
                    )
                return g_sb

            def out_phase(og, g_sb):
                """Run the scores @ gT matmuls for a direct-order o-group."""
                osl = slice(og * OG, (og + 1) * OG)
                for ct in range(NDT):
                    o_ps = po.tile([128, OG], f32, name="o_ps", tag="so")
                    for dt_ in range(NDT):
                        nc.tensor.matmul(
                            o_ps[:],
                            lhsT=sT_sb[:, dt_, ct * 128 : (ct + 1) * 128],
                            rhs=g_sb[:, dt_, :],
                            start=(dt_ == 0),
                            stop=(dt_ == NDT - 1),
                        )
                    out_t = outp.tile([128, OG], f32, name="out_t")
                    nc.vector.tensor_copy(out_t[:], o_ps[:])
                    nc.sync.dma_start(out=out_r[ct, :, osl], in_=out_t[:])

            def bog_phase(og):
                """Reassociated o-group: out[:, og] = uT^T W3T + r b3^T."""
                osl = slice(og * OG, (og + 1) * OG)
                b3_t = b3_tile(og)
                # rank-1 bias term precomputed on DVE; consumed by the
                # PSUM evacuation adds at the end of the stream
                tmp_l = []
                for ct in range(NDT):
                    tmp_t = tmpp.tile([128, OG], bf16, name="tmp_t")
                    nc.vector.tensor_scalar_mul(
                        tmp_t[:], b3_t[:], r_sb[:, ct : ct + 1]
                    )
                    tmp_l.append(tmp_t)
                ps_l = [pg.tile([128, OG], f32, name="g_ps") for _ in range(NDT)]
                n0 = 0
                for ch in [NCH] * NB:
                    w3_t = w3p.tile([128, NCH, OG], bf16, name="w3_t")
                    nc.sync.dma_start(
                        out=w3_t[:, :ch, :], in_=w3_r[:, og, n0 : n0 + ch, :]
                    )
                    for j in range(ch):
                        for ct in range(NDT):
                            nc.tensor.matmul(
                                ps_l[ct][:],
                                lhsT=ut_sb[:, n0 + j, ct * 128 : (ct + 1) * 128],
                                rhs=w3_t[:, j, :],
                                start=(n0 + j == 0),
                                stop=(n0 + j == NI - 1),
                            )
                    n0 += ch
                for ct in range(NDT):
                    out_t = outp.tile([128, OG], f32, name="out_t")
                    nc.vector.tensor_add(out_t[:], ps_l[ct][:], tmp_l[ct][:])
                    nc.sync.dma_start(out=out_r[ct, :, osl], in_=out_t[:])

            # o-group 0's g-stream first, with x1 loads interleaved: the PE
            # starts as soon as the first x1/W3 tile pair lands.
            g0 = g_phase(0, x_load=(x1_sb, x1_r))  # x1 rides og0's stream
            b3_t0 = b3_tile(0)

            # W1T/W2T zero-padded on-chip to 128 output columns: M=128
            # matmuls get fast weight load (216ns vs 592ns measured at
            # M=16), while only 160KB each moves over DMA.
            w1t_sb = persist.tile([128, NI, 128], bf16, name="w1t_sb")
            nc.vector.memset(w1t_sb[:], 0.0)
            nc.sync.dma_start(out=w1t_sb[:, :, :F], in_=w1_r[:])
            w2t_sb = persist.tile([128, NI, 128], bf16, name="w2t_sb")
            nc.vector.memset(w2t_sb[:], 0.0)
            nc.sync.dma_start(out=w2t_sb[:, :, :F], in_=w2_r[:])
            b1_sb = persist.tile([F, 1], f32, name="b1_sb")
            nc.sync.dma_start(out=b1_sb[:], in_=b1[:])
            b2_sb = persist.tile([F, 1], f32, name="b2_sb")
            nc.sync.dma_start(out=b2_sb[:], in_=b2[:])
            ones_sb = persist.tile([128, 1], bf16, name="ones_sb")
            nc.vector.memset(ones_sb[:], 1.0)

            # ---- q = W1 @ x1 + b1 -> [F, C] fp32 ----
            q_ps = pqk.tile([128, C], f32, name="q_ps", tag="qk")
            for n in range(NI):
                nc.tensor.matmul(
                    q_ps[:],
                    lhsT=w1t_sb[:, n, :],
                    rhs=x1_sb[:, n, :],
                    start=(n == 0),
                    stop=(n == NI - 1),
                )
            q_sb = persist.tile([F, C], f32, name="q_sb")
            nc.vector.tensor_scalar_add(q_sb[:], q_ps[:F, :], b1_sb[:])

            # og0's gT evacuates now (Vector is idle; g0 psum is complete)
            g_sb0 = evac_phase(g0, b3_t0)

            # o-group 1's g-stream carries the x2 loads (k runs after it)
            g1 = g_phase(1, x_load=(x2_sb, x2_r))
            b3_t1 = b3_tile(1)

            # ---- k = W2 @ x2 + b2 -> [F, C] fp32 ----
            k_ps = pqk.tile([128, C], f32, name="k_ps", tag="qk")
            for n in range(NI):
                nc.tensor.matmul(
                    k_ps[:],
                    lhsT=w2t_sb[:, n, :],
                    rhs=x2_sb[:, n, :],
                    start=(n == 0),
                    stop=(n == NI - 1),
                )
            k_sb = persist.tile([F, C], f32, name="k_sb")
            nc.vector.tensor_scalar_add(k_sb[:], k_ps[:F, :], b2_sb[:])

            # og1's gT evacuates immediately too
            g_sb1 = evac_phase(g1, b3_t1)

            # x1T streams in while softmax/out-phases run; the u-phase
            # consumes it granule by granule.
            for gch in range(NB):
                nc.sync.dma_start(
                    out=x1t_sb[:, gch * NCH : (gch + 1) * NCH, :, :],
                    in_=x1t_r[:, gch * NCH : (gch + 1) * NCH, :, :],
                )

            # ---- sT[d, c] = sum_f k[f,d] q[f,c] (plain fp32 matmul),
            #      then softmax over free (c); emit bf16 scores.  The tiny
            #      r-matmuls (r[c] = sum_d sT[d,c], partition reduce via a
            #      ones vector) interleave into the softmax window so the
            #      PE has work while Scalar/Vector normalize. ----
            # four separate PSUM tiles: a column-sliced accumulation in one
            # bank corrupts sibling columns (start=True resets the bank).
            # pg-ring slots are free here since the g evacuations ran early.
            r_ps_l = [pg.tile([128, 1], f32, name="g_ps") for _ in range(NDT)]

            def softmax_tail(dt_, s_ps):
                # logits are bounded (|s| < ~10 for this problem), so plain
                # exp is fp32-safe; skipping the max keeps Exp at one sync
                # wait (the Activation ISA slot allows only one).
                e_sb = small.tile([128, C], f32, name="e_sb")
                esum = small.tile([128, 1], f32, name="esum")
                nc.scalar.activation(
                    e_sb[:], s_ps[:], AF.Exp, scale=1.0, accum_out=esum[:],
                )
                rcp = small.tile([128, 1], f32, name="rcp")
                nc.vector.reciprocal(rcp[:], esum[:])
                nc.vector.tensor_scalar_mul(sT_sb[:, dt_, :], e_sb[:], rcp[:])
                for ct in range(NDT):
                    nc.tensor.matmul(
                        r_ps_l[ct][:],
                        lhsT=sT_sb[:, dt_, ct * 128 : (ct + 1) * 128],
                        rhs=ones_sb[:],
                        start=(dt_ == 0),
                        stop=(dt_ == NDT - 1),
                    )

            s_pend = None
            for dt_ in range(NDT):
                s_ps = po.tile([128, C], f32, name="s_ps", tag="so")
                nc.tensor.matmul(
                    s_ps[:],
                    lhsT=k_sb[:, dt_ * 128 : (dt_ + 1) * 128],
                    rhs=q_sb[:],
                    start=True,
                    stop=True,
                )
                if s_pend is not None:
                    softmax_tail(dt_ - 1, s_pend)
                s_pend = s_ps
            softmax_tail(NDT - 1, s_pend)
            r_sb = persist.tile([128, NDT], f32, name="r_sb")
            for ct in range(NDT):
                nc.vector.tensor_copy(r_sb[:, ct : ct + 1], r_ps_l[ct][:])

            # ---- direct-order output for o-groups 0/1 ----
            out_phase(0, g_sb0)
            out_phase(1, g_sb1)

            # ---- u-phase: uT[i, c] = sum_d x1[i,d] sT[d,c], bf16 ----
            for n in range(NI):
                u_ps = po.tile([128, C], f32, name="u_ps", tag="so")
                for dt_ in range(NDT):
                    nc.tensor.matmul(
                        u_ps[:],
                        lhsT=x1t_sb[:, n, dt_, :],
                        rhs=sT_sb[:, dt_, :],
                        start=(dt_ == 0),
                        stop=(dt_ == NDT - 1),
                    )
                nc.vector.tensor_copy(ut_sb[:, n, :], u_ps[:])

            # ---- main: reassociated stream for o-groups 2..15 ----
            for og in range(2, N_OG):
                bog_phase(og)

    nc.finalize()
    return nc


def _get_nc():
    if "nc" not in _NC_CACHE:
        _NC_CACHE["nc"] = _build_nc()
    return _NC_CACHE["nc"]


def _stage_inputs(input1, input2, W1, b1, W2, b2, W3, b3):
    input1 = np.asarray(input1, np.float32)
    input2 = np.asarray(input2, np.float32)
    W1 = np.asarray(W1, np.float32)
    W2 = np.asarray(W2, np.float32)
    W3 = np.asarray(W3, np.float32)
    b1 = np.asarray(b1, np.float32)
    b2 = np.asarray(b2, np.float32)
    b3 = np.asarray(b3, np.float32)

    def pmajor(X, inner):
        # [THW, inner] -> [128, NI*inner]: row p = concat_n X[n*128+p, :],
        # so every DMA chunk is a contiguous multi-KB run per partition
        return np.ascontiguousarray(
            X.reshape(NI, 128, inner).transpose(1, 0, 2).reshape(128, NI * inner)
        )

    # [B,T,C,H,W] -> x[b][i=(t,hw), c], bf16, partition-major
    X1 = np.ascontiguousarray(
        input1.reshape(B, T, C, H * W).transpose(0, 1, 3, 2)
    ).reshape(B, THW, C).astype(BF16NP)
    X2 = np.ascontiguousarray(
        input2.reshape(B, T, C, H * W).transpose(0, 1, 3, 2)
    ).reshape(B, THW, C).astype(BF16NP)
    X1p = [pmajor(X1[b], C) for b in range(B)]
    X2p = [pmajor(X2[b], C) for b in range(B)]
    # channel-major x1: [128p, (n, dt, j)] with value x1[n*128+j, dt*128+p]
    X1Tp = [
        np.ascontiguousarray(
            X1[b].reshape(NI, 128, NDT, 128).transpose(3, 0, 2, 1)
        ).reshape(128, NI * NDT * 128)
        for b in range(B)
    ]
    W1Tp = pmajor(np.ascontiguousarray(W1.T).astype(BF16NP), F)
    W2Tp = pmajor(np.ascontiguousarray(W2.T).astype(BF16NP), F)
    W3T = np.ascontiguousarray(W3.T).astype(BF16NP)   # [THW, O_TOT]
    b1c = np.ascontiguousarray(b1.reshape(F, 1))
    b2c = np.ascontiguousarray(b2.reshape(F, 1))
    b3h = b3.astype(BF16NP)

    in_maps = []
    for core in range(8):
        b = core // 2
        half = core % 2
        osl = slice(half * O_HALF, (half + 1) * O_HALF)
        # [THW, O_HALF] -> [128, (og, n, oc)]
        w3p_core = np.ascontiguousarray(
            W3T[:, osl]
            .reshape(NI, 128, N_OG, OG)
            .transpose(1, 2, 0, 3)
            .reshape(128, N_OG * NI * OG)
        )
        in_maps.append(
            {
                "x1": X1p[b],
                "x2": X2p[b],
                "x1t": X1Tp[b],
                "w1t": W1Tp,
                "w2t": W2Tp,
                "w3t": w3p_core,
                "b1": b1c,
                "b2": b2c,
                "b3": np.ascontiguousarray(
                    np.broadcast_to(b3h[osl][None, :], (128, O_HALF))
                ),
            }
        )
    return in_maps


def run(inputs: dict, trace: bool = False):
    """Returns (full_output [B,F,C,H,W], BassKernelResults)."""
    in_maps = _stage_inputs(**inputs)
    nc = _get_nc()
    res = run_bass_kernel_spmd(nc, in_maps, core_ids=list(range(8)), trace=trace)
    out_full = np.empty((B, C, O_TOT), np.float32)
    for core in range(8):
        b = core // 2
        half = core % 2
        out_full[b, :, half * O_HALF : (half + 1) * O_HALF] = res.results[core]["out"]
    out = np.ascontiguousarray(
        out_full.reshape(B, C, F, H, W).transpose(0, 2, 1, 3, 4)
    )
    return out, res


def kernel(**inputs) -> np.ndarray:
    out, _ = run(inputs, trace=False)
    return out
